# revision 38
# baseline (speedup 1.0000x reference)
"""BigBird block-sparse attention on 8 Trainium2 NeuronCores (Bass/Tile).

Shapes (hardcoded): B=2, H=12, S=4096, D=64, block=64 -> nb=64 blocks, nw=62.
Sharding: 24 (b,h) pairs -> 3 per core (batch x head parallel, SPMD).

Device math per (b,h) pair, scores-TRANSPOSED orientation (keys on PSUM
partitions) so that exp(scores^T) is directly the lhsT of the context matmul:

  sparse blocks l=1..62: 4 score matmuls  S^T[128k, 64q] per 128-key chunk:
      chunk0 = [kblock0 | kblock63]   (global)
      chunk1 = [l-1 | l] (or [1|2] for l=1, [61|62] for l=62)  (window, from KT)
      chunk2 = [l+1 or pad | r0]      (staged)
      chunk3 = [r1 | r2]              (staged)
  exp (ACT, scale=1/sqrt(64), batched over groups of 6 blocks)
  4 ctx matmuls: lhsT = A^T chunk [128k, 64q], rhs = V chunk [128k, 65]
      (65th V column is 1.0 for real keys / 0.0 for pad keys -> col 64 of the
       PSUM result is the softmax denominator; pad keys contribute nothing)
  out rows = ctx[:, :64] * recip(ctx[:, 64])

  dense blocks 0 and 63: key-chunk loop over all 32 chunks of 128 keys,
  rhs = QT columns of q-blocks {0, 63}; same exp + ctx + ones-column scheme.
"""

import os
import sys
import numpy as np

sys.path.insert(0, "/opt/trn_rl_repo")

import ml_dtypes

B, H, S, D = 2, 12, 4096, 64
BLK = 64
NB = S // BLK          # 64
NW = NB - 2            # 62
R = 3
NCORES = 8
PAIRS_PER_CORE = (B * H) // NCORES  # 3
SCALE = 1.0 / (D ** 0.5)
GROUP = 6              # sparse blocks per exp batch (3 PSUM banks)

_BF16 = np.float16  # fp16: 3 more mantissa bits than bf16 at identical PE speed


def _np(x):
    return np.asarray(x)


def _es(spec, *ops):
    return np.einsum(spec, *ops, optimize=True)


def _ref_numpy(query, key, value, q_mask, kv_mask, band_mask, q_block_mask,
               kv_block_mask, random_attn, q_block_size, kv_block_size):
    """Plain numpy port of reference.py (fallback for non-default masks)."""
    Bq, Hq, Sq, Dq = query.shape
    qb, kb = int(q_block_size), int(kv_block_size)
    nb, nkb = Sq // qb, Sq // kb
    scale = 1.0 / (Dq ** 0.5)

    def masked(s, m):
        return np.where(m == 0, -np.inf, s)

    def softmax(s):
        m = np.max(s, axis=-1, keepdims=True)
        e = np.exp(s - m)
        return e / np.sum(e, axis=-1, keepdims=True)

    ra = np.broadcast_to(random_attn[None].astype(np.int64),
                         (Bq,) + random_attn.shape)
    nw, r = ra.shape[2], ra.shape[3]
    bidx = np.arange(Bq)[:, None, None, None]
    hidx = np.arange(Hq)[None, :, None, None]
    rm = kv_block_mask[bidx, ra].reshape(Bq, Hq, nw, r * kb)
    random_mask = _es('blq,bhlk->bhlqk', q_block_mask[:, 1:-1], rm)

    bq = query.reshape(Bq, Hq, nb, qb, Dq)
    bk = key.reshape(Bq, Hq, nkb, kb, Dq)
    bv = value.reshape(Bq, Hq, nkb, kb, Dq)
    sk = bk[bidx, hidx, ra].reshape(Bq, Hq, nw, r * kb, Dq)
    sv = bv[bidx, hidx, ra].reshape(Bq, Hq, nw, r * kb, Dq)

    p1 = _es('bhqd,bhkd->bhqk', bq[:, :, 0], key) * scale
    a1 = softmax(masked(p1, kv_mask))
    c1 = _es('bhqk,bhkd->bhqd', a1, value)[:, :, None]

    k2 = np.concatenate([bk[:, :, 0], bk[:, :, 1], bk[:, :, 2], bk[:, :, -1],
                         sk[:, :, 0]], axis=2)
    v2 = np.concatenate([bv[:, :, 0], bv[:, :, 1], bv[:, :, 2], bv[:, :, -1],
                         sv[:, :, 0]], axis=2)
    p2 = _es('bhqd,bhkd->bhqk', bq[:, :, 1], k2) * scale
    seq_pad = np.concatenate([kv_mask[:, :, :, :3 * kb], kv_mask[:, :, :, -kb:],
                              np.ones_like(random_mask[:, :1, 0, :1])], axis=3)
    rand_pad = np.concatenate([np.ones_like(p2[:, :, :, :4 * kb]),
                               random_mask[:, :, 0]], axis=3)
    a2 = softmax(masked(p2, np.minimum(seq_pad, rand_pad)))
    c2 = _es('bhqk,bhkd->bhqd', a2, v2)[:, :, None]

    ebk = np.concatenate([bk[:, :, 1:-3], bk[:, :, 2:-2], bk[:, :, 3:-1]], axis=3)
    ebv = np.concatenate([bv[:, :, 1:-3], bv[:, :, 2:-2], bv[:, :, 3:-1]], axis=3)
    mq = bq[:, :, 2:-2]
    inner = masked(_es('bhlqd,bhlkd->bhlqk', mq, ebk) * scale, band_mask)
    randp = masked(_es('bhlqd,bhlkd->bhlqk', mq, sk[:, :, 1:-1]) * scale,
                   random_mask[:, :, 1:-1])
    fop = masked(_es('bhlqd,bhkd->bhlqk', mq, bk[:, :, 0]) * scale,
                 kv_mask[:, :, :, :kb][:, :, :, None, :])
    lop = masked(_es('bhlqd,bhkd->bhlqk', mq, bk[:, :, -1]) * scale,
                 kv_mask[:, :, :, -kb:][:, :, :, None, :])
    band = np.concatenate([fop, inner, lop, randp], axis=-1)
    aw = softmax(band)
    cm = _es('bhlqk,bhlkd->bhlqd', aw[..., kb:4 * kb], ebv)
    cm += _es('bhlqk,bhlkd->bhlqd', aw[..., 4 * kb:-kb], sv[:, :, 1:-1])
    cm += _es('bhlqk,bhkd->bhlqd', aw[..., :kb], bv[:, :, 0])
    cm += _es('bhlqk,bhkd->bhlqd', aw[..., -kb:], bv[:, :, -1])

    k3 = np.concatenate([bk[:, :, 0], bk[:, :, -3], bk[:, :, -2], bk[:, :, -1],
                         sk[:, :, -1]], axis=2)
    v3 = np.concatenate([bv[:, :, 0], bv[:, :, -3], bv[:, :, -2], bv[:, :, -1],
                         sv[:, :, -1]], axis=2)
    p3 = _es('bhqd,bhkd->bhqk', bq[:, :, -2], k3) * scale
    seq_pad3 = np.concatenate([kv_mask[:, :, :, :kb], kv_mask[:, :, :, -3 * kb:],
                               np.ones_like(random_mask[:, :1, 0, :1])], axis=3)
    rand_pad3 = np.concatenate([np.ones_like(p3[:, :, :, :4 * kb]),
                                random_mask[:, :, -1]], axis=3)
    a3 = softmax(masked(p3, np.minimum(seq_pad3, rand_pad3)))
    c3 = _es('bhqk,bhkd->bhqd', a3, v3)[:, :, None]

    p4 = _es('bhqd,bhkd->bhqk', bq[:, :, -1], key) * scale
    a4 = softmax(masked(p4, kv_mask))
    c4 = _es('bhqk,bhkd->bhqd', a4, value)[:, :, None]

    ctx = np.concatenate([c1, c2, cm, c3, c4], axis=2)
    return (ctx.reshape(Bq, Hq, Sq, Dq) * q_mask).astype(np.float32)


def _window_cols(l):
    """(start_block, chunk3_first_block_or_None) for sparse q-block l."""
    if l == 1:
        return 1, None      # window chunk = [b1 | b2], staged slot0 = pad
    if l == NW:              # l == 62
        return NW - 1, None  # [b61 | b62], staged slot0 = pad
    return l - 1, l + 1      # [l-1 | l], staged slot0 = b_{l+1}


def _chunk23_blocks(l, ra_h):
    """Score key blocks for chunks 2,3 of sparse q-block l: [x, r0, r1, r2].
    x is None for edge blocks (l=1, 62): that slot is a pad (V rows zeroed)."""
    w = l - 1
    _, extra = _window_cols(l)
    r0, r1, r2 = (int(ra_h[w, j]) for j in range(R))
    return [extra, r0, r1, r2]


def _stage_core_inputs(q, k, v, ra, pairs):
    """Build all host-staged arrays for one core (list of (b,h) pairs).

    All staged arrays are SBUF-partition-major with contiguous free dims,
    so every DMA is one large contiguous descriptor per partition."""
    P = len(pairs)
    FW = BLK + 1
    QT = np.empty((P, D, S), dtype=_BF16)
    KT = np.empty((P, D, S), dtype=_BF16)
    VA = np.empty((P, 128, 32 * FW), dtype=_BF16)   # V pairs (2c,2c+1) + ones
    VB = np.empty((P, 128, 31 * FW), dtype=_BF16)   # V pairs (2c+1,2c+2)
    VG2 = np.empty((P, 128, NW * 2 * FW), dtype=_BF16)  # [x|r0],[r1|r2] pairs
    KTr = np.empty((P, D, NW * 4 * BLK), dtype=_BF16)   # [x|r0|r1|r2] cols
    KTd0 = np.empty((P, D, 128), dtype=_BF16)             # [b0 | b63]
    QTd = np.empty((P, D, 128), dtype=_BF16)              # [q0 | q63]
    Vd0 = np.empty((P, 128, FW), dtype=_BF16)

    for i, (b, h) in enumerate(pairs):
        Q = q[b, h]; K = k[b, h]; V = v[b, h]
        qt = Q.T.astype(_BF16); kt = K.T.astype(_BF16)
        QT[i] = qt; KT[i] = kt
        vv = np.concatenate([V, np.ones((S, 1), np.float32)], 1).astype(_BF16)
        VA[i] = vv.reshape(32, 128, FW).transpose(1, 0, 2).reshape(128, -1)
        VB[i] = vv[BLK:-BLK].reshape(31, 128, FW).transpose(1, 0, 2) \
            .reshape(128, -1)
        QTd[i, :, :BLK] = qt[:, :BLK]
        QTd[i, :, BLK:] = qt[:, -BLK:]
        Vd0[i, :BLK] = vv[:BLK]
        Vd0[i, BLK:] = vv[-BLK:]
        vg = np.zeros((NW, 2, 2 * BLK, FW), np.float32)
        for l in range(1, NB - 1):
            w = l - 1
            x, r0, r1, r2 = _chunk23_blocks(l, ra[h])
            # reference's middle-block V rotation: score(k63)*V[r0],
            # score(r0)*V[r1], score(r1)*V[r2], score(r2)*V[63].
            # chunk regrouping: c0=[k0|r2]->(V0|V63)=vd0,
            # c2=[l+1|k63]->(V_{l+1}|V_r0) (K via stepped kt AP),
            # c3=[r0|r1]->(V_r1|V_r2).  Edge blocks keep the straight map:
            # c0=[k0|k63]->vd0, c2=[pad|r0]->(0|V_r0) (K_r0 via ktd0),
            # c3=[r1|r2]->(V_r1|V_r2).
            if x is None:
                kc0 = (0, NB - 1)
                kc3 = (r1, r2)
                vc2 = (None, r0)
                vc3 = (r1, r2)
                KTd0[i, :, (0 if w == 0 else 1) * BLK:
                     (1 if w == 0 else 2) * BLK] = \
                    kt[:, r0 * BLK:(r0 + 1) * BLK]
            else:
                kc0 = (0, r2)
                kc3 = (r0, r1)
                vc2 = (x, r0)
                vc3 = (r1, r2)
            for s_i, blkid in enumerate(kc0):
                KTr[i, :, w * 4 * BLK + s_i * BLK:
                    w * 4 * BLK + (s_i + 1) * BLK] = \
                    kt[:, blkid * BLK:(blkid + 1) * BLK]
            for s_i, blkid in enumerate(kc3):
                KTr[i, :, w * 4 * BLK + (2 + s_i) * BLK:
                    w * 4 * BLK + (3 + s_i) * BLK] = \
                    kt[:, blkid * BLK:(blkid + 1) * BLK]
            for ci, pair in enumerate((vc2, vc3)):
                for s_i, blkid in enumerate(pair):
                    if blkid is not None:
                        vg[w, ci, s_i * BLK:(s_i + 1) * BLK, :BLK] = \
                            V[blkid * BLK:(blkid + 1) * BLK]
                        vg[w, ci, s_i * BLK:(s_i + 1) * BLK, BLK] = 1.0
        VG2[i] = vg.astype(_BF16).transpose(2, 0, 1, 3).reshape(128, -1)
    dumA = np.zeros((P, D, 16), dtype=_BF16)
    dumB = np.zeros((P, 128, 16), dtype=_BF16)
    blobA = np.concatenate([dumA, QT, KT, KTr, KTd0, QTd], axis=2)
    blobB = np.concatenate([dumB, VA, VB, VG2, Vd0], axis=2)
    return dict(blobA=blobA, blobB=blobB)


def _build_program():
    import concourse.bass as bass
    import concourse.bacc as bacc
    import concourse.tile as tile
    from concourse import mybir

    bf16 = mybir.dt.float16  # fp16 everywhere (see _BF16)
    f32 = mybir.dt.float32
    EXP = mybir.ActivationFunctionType.Exp
    P3 = 1   # one (b,h) pair per program invocation (3 waves per core)
    FW = BLK + 1
    # Bacc (not plain Bass): its finalize() runs move_matmul_waits_to_
    # ldweights + generate_event_semaphores, which split multi-waits to
    # satisfy this walrus build's 1-wait-per-instruction constraint.
    nc = bacc.Bacc()
    # blobA (64 partitions): [QT | KT | KTr | KTd0 | QTd]
    # blobB (128 partitions): [VA | VB | VG2 | Vd0]
    # 16 dummy head columns: pair-0/1 DMAs write them (the pair-0 observer
    # DMAs read them); pair-2's DMA skips them, avoiding a WAR with the
    # observers that would add a second wait.
    CA = (S, S, NW * 4 * BLK, 128, 128)
    OA = [16]
    for c in CA:
        OA.append(OA[-1] + c)
    CB = (32 * FW, 31 * FW, NW * 2 * FW, FW)
    OB = [16]
    for c in CB:
        OB.append(OB[-1] + c)
    blobAp = nc.declare_dram_parameter("blobA", [P3, D, OA[-1]], bf16,
                                       isOutput=False)
    blobBp = nc.declare_dram_parameter("blobB", [P3, 128, OB[-1]], bf16,
                                       isOutput=False)
    outp = nc.declare_dram_parameter("out", [P3, BLK, NB * BLK], bf16,
                                     isOutput=True)
    ngroups = (NW + GROUP - 1) // GROUP  # 11 groups: 10x6 + 1x2

    with tile.TileContext(nc) as tc:
        with (
            tc.tile_pool(name="big", bufs=1) as big,      # per-pair inputs
            tc.tile_pool(name="outp", bufs=1) as outpool,  # fresh out slot
            tc.tile_pool(name="at", bufs=2) as atp,       # exp outputs
            tc.tile_pool(name="small", bufs=2) as small,
            tc.tile_pool(name="ps_s", bufs=2, space="PSUM") as ps_s,
            tc.tile_pool(name="ps_c", bufs=2, space="PSUM") as ps_c,
        ):
            # walrus codegen allows at most ONE semaphore wait per engine
            # instruction.  Every SBUF input tile gets a fresh slot (bufs=3,
            # one per pair) so DMAs carry no waits; before each phase the
            # remaining cross-engine deps are staged one-at-a-time onto
            # 1-column ldweights (PE), a self-copy (ACT), and one toucher
            # matmul (PE self-release of recycled PSUM slots), with
            # no_sync_barrier() pinning segment order.
            prev_at = None   # last SBUF tile written by ACT (bf16)
            prev_scr = None  # last SBUF tile written by DVE (bf16)
            for p in range(P3):
                # one pair per program: every SBUF tile is written exactly
                # once, so no DMA carries a slot-release or WAW wait; the
                # only dep-carrying DMA is the output (1 DVE wait, first on
                # its HWDGE queue).  The host invokes this program once per
                # pair (3 waves).
                tc.no_sync_barrier()
                ba = big.tile([D, OA[-1]], bf16, tag="ba")
                nc.sync.dma_start(out=ba[:], in_=blobAp[p])
                tc.no_sync_barrier()
                bb = big.tile([128, OB[-1]], bf16, tag="bb")
                nc.sync.dma_start(out=bb[:], in_=blobBp[p])
                tc.no_sync_barrier()
                qt = ba[:, OA[0]:OA[1]]
                kt = ba[:, OA[1]:OA[2]]
                ktr = ba[:, OA[2]:OA[3]]
                ktd0 = ba[:, OA[3]:OA[4]]
                qtd = ba[:, OA[4]:OA[5]]
                va = bb[:, OB[0]:OB[1]]
                vb = bb[:, OB[1]:OB[2]]
                vg2 = bb[:, OB[2]:OB[3]]
                vd0 = bb[:, OB[3]:OB[4]]
                # +1 scratch column: the DVE slot-acquire self-copy below
                # writes garbage there, outside the DMA'd output region.
                out_sb = outpool.tile([BLK, NB * BLK + 1], bf16, tag="out")

                # absorb the two blob-DMA completion waits, one per PE
                # inst; the DVE self-copy acquires out_sb's slot so the
                # out-DMA release wait lands here, not on the first real DVE
                # write (the garbage cell is overwritten later).
                tc.no_sync_barrier()
                nc.tensor.ldweights(ba[0:64, 0:1])
                nc.tensor.ldweights(bb[0:64, 0:1])
                nc.vector.tensor_copy(out_sb[0:1, NB * BLK:NB * BLK + 1],
                                      out_sb[0:1, 0:1])

                # ---- sparse q-blocks 1..62, in groups of GROUP ----
                for g in range(ngroups):
                    w0 = g * GROUP
                    ng = min(GROUP, NW - w0)
                    sps = ps_s.tile([128, GROUP * 256], f32, tag="s")
                    ctile = ps_c.tile([BLK, GROUP * FW + 2], f32, tag="c")
                    sc = GROUP * FW
                    tc.no_sync_barrier()
                    if prev_at is not None:
                        nc.tensor.ldweights(prev_at[0:64, 0:1])
                    if prev_scr is not None:
                        nc.tensor.ldweights(prev_scr[0:64, 0:1])
                    if prev_at is not None:
                        ascr = small.tile([1, 1], bf16, tag="ascr")
                        nc.scalar.copy(ascr[:], prev_at[0:1, 0:1])
                    tc.no_sync_barrier()
                    nc.tensor.matmul(out=ctile[0:1, sc:sc + 1],
                                     lhsT=qt[0:64, 0:1], rhs=qt[0:64, 0:1],
                                     start=True, stop=True)
                    tc.no_sync_barrier()
                    for j in range(ng):
                        l = 1 + w0 + j
                        ws, _ = _window_cols(l)
                        qcols = qt[:, l * BLK:(l + 1) * BLK]
                        base = j * 256
                        kc = (l - 1) * 4 * BLK
                        # c0: staged [K_k0|K_r2] (mid) / [K_k0|K_k63] (edge)
                        nc.tensor.matmul(
                            out=sps[:, base:base + BLK],
                            lhsT=ktr[:, kc:kc + 2 * BLK],
                            rhs=qcols, start=True, stop=True)
                        # c1: window pair from kt (contiguous)
                        nc.tensor.matmul(
                            out=sps[:, base + BLK:base + 2 * BLK],
                            lhsT=kt[:, ws * BLK:(ws + 2) * BLK],
                            rhs=qcols, start=True, stop=True)
                        # c2: mid = stepped [K_{l+1}|K_k63]; edge = dup-score
                        # top half (V zeroed) + staged K_r0 bottom half
                        if 1 < l < NW:
                            nc.tensor.matmul(
                                out=sps[0:64, base + 2 * BLK:base + 3 * BLK],
                                lhsT=kt[:, (l + 1) * BLK:(l + 2) * BLK],
                                rhs=qcols, start=True, stop=True)
                            nc.tensor.matmul(
                                out=sps[64:128, base + 2 * BLK:base + 3 * BLK],
                                lhsT=kt[:, (NB - 1) * BLK:NB * BLK],
                                rhs=qcols, start=True, stop=True)
                        else:
                            nc.tensor.matmul(
                                out=sps[0:64, base + 2 * BLK:base + 3 * BLK],
                                lhsT=kt[:, l * BLK:(l + 1) * BLK],
                                rhs=qcols, start=True, stop=True)
                            eh = 0 if l == 1 else 1
                            nc.tensor.matmul(
                                out=sps[64:128, base + 2 * BLK:base + 3 * BLK],
                                lhsT=ktd0[:, eh * BLK:(eh + 1) * BLK],
                                rhs=qcols, start=True, stop=True)
                        # c3: staged [K_r0|K_r1] (mid) / [K_r1|K_r2] (edge)
                        nc.tensor.matmul(
                            out=sps[:, base + 3 * BLK:base + 4 * BLK],
                            lhsT=ktr[:, kc + 2 * BLK:kc + 4 * BLK],
                            rhs=qcols, start=True, stop=True)

                    at = atp.tile([128, GROUP * 256], bf16, tag="at")
                    nc.scalar.activation(at[:, :ng * 256], sps[:, :ng * 256],
                                         EXP, scale=SCALE)

                    for j in range(ng):
                        l = 1 + w0 + j
                        ws, _ = _window_cols(l)
                        w = l - 1
                        base = j * 256
                        if ws % 2 == 0:
                            vwin = va[:, (ws // 2) * FW:(ws // 2 + 1) * FW]
                        else:
                            vwin = vb[:, ((ws - 1) // 2) * FW:
                                      ((ws - 1) // 2 + 1) * FW]
                        rhs = [vd0[:], vwin,
                               vg2[:, (2 * w) * FW:(2 * w + 1) * FW],
                               vg2[:, (2 * w + 1) * FW:(2 * w + 2) * FW]]
                        for c in range(4):
                            nc.tensor.matmul(
                                out=ctile[:, j * FW:(j + 1) * FW],
                                lhsT=at[:, base + c * BLK: base + (c + 1) * BLK],
                                rhs=rhs[c], start=(c == 0), stop=(c == 3))
                    csb = small.tile([BLK, GROUP * FW], f32, tag="csb")
                    nc.vector.tensor_copy(csb[:, :ng * FW], ctile[:, :ng * FW])
                    for j in range(ng):
                        l = 1 + w0 + j
                        rec = small.tile([BLK, 1], f32, tag="rec")
                        nc.vector.reciprocal(
                            rec[:], csb[:, j * FW + BLK: j * FW + BLK + 1])
                        nc.vector.tensor_scalar_mul(
                            out_sb[:, l * BLK:(l + 1) * BLK],
                            csb[:, j * FW: j * FW + BLK], rec[:, 0:1])
                    # bf16 DVE anchor: ldweights of this absorbs the DVE tick
                    scr = small.tile([BLK, 1], bf16, tag="scr")
                    nc.vector.tensor_copy(scr[:], csb[:, 0:1])
                    prev_at, prev_scr = at, scr

                # ---- dense q-blocks 0 and 63: 32 key chunks in 3 rounds ----
                cdense2 = ps_c.tile([128, FW + 2], f32, tag="c")
                tc.no_sync_barrier()
                nc.tensor.ldweights(prev_at[0:64, 0:1])
                nc.tensor.ldweights(prev_scr[0:64, 0:1])
                ascr = small.tile([1, 1], bf16, tag="ascr")
                nc.scalar.copy(ascr[:], prev_at[0:1, 0:1])
                tc.no_sync_barrier()
                nc.tensor.matmul(
                    out=cdense2[0:1, FW:FW + 1],
                    lhsT=va[0:64, 0:1], rhs=va[0:64, 0:1],
                    start=True, stop=True)
                tc.no_sync_barrier()
                # both dense blocks in ONE accumulation group: out
                # partitions 0-63 = q0 rows, 64-127 = q63 rows.  (Two
                # interleaved open groups in one PSUM bank corrupt.)
                cd = cdense2
                CH_PER = 12
                done = 0
                for rnd in range(3):
                    nch = min(CH_PER, 32 - done)
                    sps = ps_s.tile([128, GROUP * 256], f32, tag="s")
                    if rnd >= 1:
                        tc.no_sync_barrier()
                        if rnd == 2:
                            nc.tensor.ldweights(prev_at[0:64, 0:1])
                        ascr = small.tile([1, 1], bf16, tag="ascr")
                        nc.scalar.copy(ascr[:], prev_at[0:1, 0:1])
                        tc.no_sync_barrier()
                    for i in range(nch):
                        cc = done + i
                        nc.tensor.matmul(
                            out=sps[:, i * 128:(i + 1) * 128],
                            lhsT=kt[:, cc * 128:(cc + 1) * 128],
                            rhs=qtd[:], start=True, stop=True)
                    at = atp.tile([128, GROUP * 256], bf16, tag="at")
                    nc.scalar.activation(at[:, :nch * 128], sps[:, :nch * 128],
                                         EXP, scale=SCALE)
                    for i in range(nch):
                        cc = done + i
                        nc.tensor.matmul(
                            out=cd[:, 0:FW],
                            lhsT=at[:, i * 128:(i + 1) * 128],
                            rhs=va[:, cc * FW:(cc + 1) * FW],
                            start=(cc == 0), stop=(cc == 31))
                    done += nch
                    prev_at = at
                csb2 = small.tile([128, FW], f32, tag="csb2")
                nc.vector.tensor_copy(csb2[:], cd[:, :FW])
                rec2 = small.tile([128, 1], f32, tag="rec2")
                nc.vector.reciprocal(rec2[:], csb2[:, BLK:BLK + 1])
                od = small.tile([128, BLK], bf16, tag="od")
                nc.vector.tensor_scalar_mul(od[:], csb2[:, :BLK], rec2[:, 0:1])
                scr = small.tile([BLK, 1], bf16, tag="scr")
                nc.vector.tensor_copy(scr[:], csb2[0:64, 0:1])
                prev_scr = scr

                nc.sync.dma_start(out=outp[p, :, BLK:(NB - 1) * BLK],
                                  in_=out_sb[:, BLK:(NB - 1) * BLK])
                nc.sync.dma_start(out=outp[p, :, 0:BLK], in_=od[0:64, :])
                nc.sync.dma_start(out=outp[p, :, (NB - 1) * BLK:NB * BLK],
                                  in_=od[64:128, :])
    nc.finalize()
    return nc


_PROGRAM = None


def kernel(**inputs) -> np.ndarray:
    q = _np(inputs["query"]).astype(np.float32)
    k = _np(inputs["key"]).astype(np.float32)
    v = _np(inputs["value"]).astype(np.float32)
    ra = _np(inputs["random_attn"]).astype(np.int64)
    masks_ok = (
        q.shape == (B, H, S, D)
        and int(_np(inputs["q_block_size"])) == BLK
        and int(_np(inputs["kv_block_size"])) == BLK
        and np.all(_np(inputs["q_mask"]) == 1)
        and np.all(_np(inputs["kv_mask"]) == 1)
        and np.all(_np(inputs["band_mask"]) == 1)
        and np.all(_np(inputs["q_block_mask"]) == 1)
        and np.all(_np(inputs["kv_block_mask"]) == 1)
    )
    if not masks_ok:
        return _ref_numpy(
            q, k, v, _np(inputs["q_mask"]).astype(np.float32),
            _np(inputs["kv_mask"]).astype(np.float32),
            _np(inputs["band_mask"]).astype(np.float32),
            _np(inputs["q_block_mask"]).astype(np.float32),
            _np(inputs["kv_block_mask"]).astype(np.float32),
            ra, int(_np(inputs["q_block_size"])),
            int(_np(inputs["kv_block_size"])))

    try:
        return _device_kernel(q, k, v, ra)
    except Exception as e:
        sys.stderr.write(f"device kernel failed ({e!r}); numpy fallback\n")
        return _ref_numpy(
            q, k, v, _np(inputs["q_mask"]).astype(np.float32),
            _np(inputs["kv_mask"]).astype(np.float32),
            _np(inputs["band_mask"]).astype(np.float32),
            _np(inputs["q_block_mask"]).astype(np.float32),
            _np(inputs["kv_block_mask"]).astype(np.float32),
            ra, BLK, BLK)


_EXEC = None


def _get_exec():
    """Module-level cached sharded executable (run_bass_via_pjrt builds a
    fresh jit closure per call, costing a full retrace per wave)."""
    global _EXEC, _PROGRAM
    if _EXEC is not None:
        return _EXEC
    import jax
    from jax.sharding import Mesh, PartitionSpec
    try:
        from jax.experimental.shard_map import shard_map
    except ImportError:
        from jax.shard_map import shard_map
    from concourse import bass2jax, mybir

    if _PROGRAM is None:
        _PROGRAM = _build_program()
    nc = _PROGRAM
    bass2jax.install_neuronx_cc_hook()
    pname = nc.partition_id_tensor.name if nc.partition_id_tensor else None
    in_names, out_names, out_avals, zero_outs = [], [], [], []
    for alloc in nc.m.functions[0].allocations:
        if not isinstance(alloc, mybir.MemoryLocationSet):
            continue
        name = alloc.memorylocations[0].name
        if alloc.kind == "ExternalInput":
            if name != pname:
                in_names.append(name)
        elif alloc.kind == "ExternalOutput":
            out_names.append(name)
            shape = tuple(alloc.tensor_shape)
            dtype = mybir.dt.np(alloc.dtype)
            out_avals.append(jax.core.ShapedArray(shape, dtype))
            zero_outs.append(np.zeros(shape, dtype))
    n_params = len(in_names)
    n_outs = len(out_avals)
    all_names = tuple(in_names + out_names
                      + ([pname] if pname is not None else []))

    def _body(*args):
        operands = list(args)
        if pname is not None:
            operands.append(bass2jax.partition_id_tensor())
        outs = bass2jax._bass_exec_p.bind(
            *operands,
            out_avals=tuple(out_avals),
            in_names=all_names,
            out_names=tuple(out_names),
            lowering_input_output_aliases=(),
            sim_require_finite=True,
            sim_require_nnan=True,
            nc=nc,
        )
        return tuple(outs)

    devices = jax.devices()[:NCORES]
    mesh = Mesh(np.asarray(devices), ("core",))
    specs = (PartitionSpec("core"),)
    fn = jax.jit(
        shard_map(_body, mesh=mesh, in_specs=specs * (n_params + n_outs),
                  out_specs=specs * n_outs, check_rep=False),
        donate_argnums=tuple(range(n_params, n_params + n_outs)),
        keep_unused=True,
    )
    _EXEC = (fn, in_names, out_names, out_avals, zero_outs)
    return _EXEC


def _device_kernel(q, k, v, ra, trace=False):
    fn, in_names, out_names, out_avals, zero_outs = _get_exec()
    pair_list = [(b, h) for b in range(B) for h in range(H)]
    out = np.empty((B, H, S, D), dtype=np.float32)
    # one program invocation per wave: core c handles pair 3c+w in wave w
    for w in range(PAIRS_PER_CORE):
        in_maps = []
        wave_pairs = []
        for c in range(NCORES):
            pr = pair_list[c * PAIRS_PER_CORE + w]
            wave_pairs.append(pr)
            in_maps.append(_stage_core_inputs(q, k, v, ra, [pr]))
        concat_in = [
            np.concatenate([in_maps[c][name] for c in range(NCORES)], axis=0)
            for name in in_names
        ]
        concat_zeros = [
            np.zeros((NCORES * z.shape[0], *z.shape[1:]), z.dtype)
            for z in zero_outs
        ]
        out_arrs = fn(*concat_in, *concat_zeros)
        o_all = np.asarray(out_arrs[out_names.index("out")]) \
            .reshape(NCORES, *out_avals[out_names.index("out")].shape)
        for c, (b, h) in enumerate(wave_pairs):
            o = o_all[c].astype(np.float32)
            # device layout [q(64), blk(64), d] -> [blk, q, d] -> [S, D]
            out[b, h] = o[0].reshape(BLK, NB, D).transpose(1, 0, 2) \
                .reshape(S, D)
    return out



# revision 39
# speedup vs baseline: 1.0185x; 1.0185x over previous
"""BigBird block-sparse attention on 8 Trainium2 NeuronCores (Bass/Tile).

Shapes (hardcoded): B=2, H=12, S=4096, D=64, block=64 -> nb=64 blocks, nw=62.
Sharding: 24 (b,h) pairs -> 3 per core (batch x head parallel, SPMD).

Device math per (b,h) pair, scores-TRANSPOSED orientation (keys on PSUM
partitions) so that exp(scores^T) is directly the lhsT of the context matmul:

  sparse blocks l=1..62: 4 score matmuls  S^T[128k, 64q] per 128-key chunk:
      chunk0 = [kblock0 | kblock63]   (global)
      chunk1 = [l-1 | l] (or [1|2] for l=1, [61|62] for l=62)  (window, from KT)
      chunk2 = [l+1 or pad | r0]      (staged)
      chunk3 = [r1 | r2]              (staged)
  exp (ACT, scale=1/sqrt(64), batched over groups of 6 blocks)
  4 ctx matmuls: lhsT = A^T chunk [128k, 64q], rhs = V chunk [128k, 65]
      (65th V column is 1.0 for real keys / 0.0 for pad keys -> col 64 of the
       PSUM result is the softmax denominator; pad keys contribute nothing)
  out rows = ctx[:, :64] * recip(ctx[:, 64])

  dense blocks 0 and 63: key-chunk loop over all 32 chunks of 128 keys,
  rhs = QT columns of q-blocks {0, 63}; same exp + ctx + ones-column scheme.
"""

import os
import sys
import numpy as np

sys.path.insert(0, "/opt/trn_rl_repo")

import ml_dtypes

B, H, S, D = 2, 12, 4096, 64
BLK = 64
NB = S // BLK          # 64
NW = NB - 2            # 62
R = 3
NCORES = 8
PAIRS_PER_CORE = (B * H) // NCORES  # 3
SCALE = 1.0 / (D ** 0.5)
GROUP = 6              # sparse blocks per exp batch (3 PSUM banks)

_BF16 = np.float16  # fp16: 3 more mantissa bits than bf16 at identical PE speed


def _np(x):
    return np.asarray(x)


def _es(spec, *ops):
    return np.einsum(spec, *ops, optimize=True)


def _ref_numpy(query, key, value, q_mask, kv_mask, band_mask, q_block_mask,
               kv_block_mask, random_attn, q_block_size, kv_block_size):
    """Plain numpy port of reference.py (fallback for non-default masks)."""
    Bq, Hq, Sq, Dq = query.shape
    qb, kb = int(q_block_size), int(kv_block_size)
    nb, nkb = Sq // qb, Sq // kb
    scale = 1.0 / (Dq ** 0.5)

    def masked(s, m):
        return np.where(m == 0, -np.inf, s)

    def softmax(s):
        m = np.max(s, axis=-1, keepdims=True)
        e = np.exp(s - m)
        return e / np.sum(e, axis=-1, keepdims=True)

    ra = np.broadcast_to(random_attn[None].astype(np.int64),
                         (Bq,) + random_attn.shape)
    nw, r = ra.shape[2], ra.shape[3]
    bidx = np.arange(Bq)[:, None, None, None]
    hidx = np.arange(Hq)[None, :, None, None]
    rm = kv_block_mask[bidx, ra].reshape(Bq, Hq, nw, r * kb)
    random_mask = _es('blq,bhlk->bhlqk', q_block_mask[:, 1:-1], rm)

    bq = query.reshape(Bq, Hq, nb, qb, Dq)
    bk = key.reshape(Bq, Hq, nkb, kb, Dq)
    bv = value.reshape(Bq, Hq, nkb, kb, Dq)
    sk = bk[bidx, hidx, ra].reshape(Bq, Hq, nw, r * kb, Dq)
    sv = bv[bidx, hidx, ra].reshape(Bq, Hq, nw, r * kb, Dq)

    p1 = _es('bhqd,bhkd->bhqk', bq[:, :, 0], key) * scale
    a1 = softmax(masked(p1, kv_mask))
    c1 = _es('bhqk,bhkd->bhqd', a1, value)[:, :, None]

    k2 = np.concatenate([bk[:, :, 0], bk[:, :, 1], bk[:, :, 2], bk[:, :, -1],
                         sk[:, :, 0]], axis=2)
    v2 = np.concatenate([bv[:, :, 0], bv[:, :, 1], bv[:, :, 2], bv[:, :, -1],
                         sv[:, :, 0]], axis=2)
    p2 = _es('bhqd,bhkd->bhqk', bq[:, :, 1], k2) * scale
    seq_pad = np.concatenate([kv_mask[:, :, :, :3 * kb], kv_mask[:, :, :, -kb:],
                              np.ones_like(random_mask[:, :1, 0, :1])], axis=3)
    rand_pad = np.concatenate([np.ones_like(p2[:, :, :, :4 * kb]),
                               random_mask[:, :, 0]], axis=3)
    a2 = softmax(masked(p2, np.minimum(seq_pad, rand_pad)))
    c2 = _es('bhqk,bhkd->bhqd', a2, v2)[:, :, None]

    ebk = np.concatenate([bk[:, :, 1:-3], bk[:, :, 2:-2], bk[:, :, 3:-1]], axis=3)
    ebv = np.concatenate([bv[:, :, 1:-3], bv[:, :, 2:-2], bv[:, :, 3:-1]], axis=3)
    mq = bq[:, :, 2:-2]
    inner = masked(_es('bhlqd,bhlkd->bhlqk', mq, ebk) * scale, band_mask)
    randp = masked(_es('bhlqd,bhlkd->bhlqk', mq, sk[:, :, 1:-1]) * scale,
                   random_mask[:, :, 1:-1])
    fop = masked(_es('bhlqd,bhkd->bhlqk', mq, bk[:, :, 0]) * scale,
                 kv_mask[:, :, :, :kb][:, :, :, None, :])
    lop = masked(_es('bhlqd,bhkd->bhlqk', mq, bk[:, :, -1]) * scale,
                 kv_mask[:, :, :, -kb:][:, :, :, None, :])
    band = np.concatenate([fop, inner, lop, randp], axis=-1)
    aw = softmax(band)
    cm = _es('bhlqk,bhlkd->bhlqd', aw[..., kb:4 * kb], ebv)
    cm += _es('bhlqk,bhlkd->bhlqd', aw[..., 4 * kb:-kb], sv[:, :, 1:-1])
    cm += _es('bhlqk,bhkd->bhlqd', aw[..., :kb], bv[:, :, 0])
    cm += _es('bhlqk,bhkd->bhlqd', aw[..., -kb:], bv[:, :, -1])

    k3 = np.concatenate([bk[:, :, 0], bk[:, :, -3], bk[:, :, -2], bk[:, :, -1],
                         sk[:, :, -1]], axis=2)
    v3 = np.concatenate([bv[:, :, 0], bv[:, :, -3], bv[:, :, -2], bv[:, :, -1],
                         sv[:, :, -1]], axis=2)
    p3 = _es('bhqd,bhkd->bhqk', bq[:, :, -2], k3) * scale
    seq_pad3 = np.concatenate([kv_mask[:, :, :, :kb], kv_mask[:, :, :, -3 * kb:],
                               np.ones_like(random_mask[:, :1, 0, :1])], axis=3)
    rand_pad3 = np.concatenate([np.ones_like(p3[:, :, :, :4 * kb]),
                                random_mask[:, :, -1]], axis=3)
    a3 = softmax(masked(p3, np.minimum(seq_pad3, rand_pad3)))
    c3 = _es('bhqk,bhkd->bhqd', a3, v3)[:, :, None]

    p4 = _es('bhqd,bhkd->bhqk', bq[:, :, -1], key) * scale
    a4 = softmax(masked(p4, kv_mask))
    c4 = _es('bhqk,bhkd->bhqd', a4, value)[:, :, None]

    ctx = np.concatenate([c1, c2, cm, c3, c4], axis=2)
    return (ctx.reshape(Bq, Hq, Sq, Dq) * q_mask).astype(np.float32)


def _window_cols(l):
    """(start_block, chunk3_first_block_or_None) for sparse q-block l."""
    if l == 1:
        return 1, None      # window chunk = [b1 | b2], staged slot0 = pad
    if l == NW:              # l == 62
        return NW - 1, None  # [b61 | b62], staged slot0 = pad
    return l - 1, l + 1      # [l-1 | l], staged slot0 = b_{l+1}


def _chunk23_blocks(l, ra_h):
    """Score key blocks for chunks 2,3 of sparse q-block l: [x, r0, r1, r2].
    x is None for edge blocks (l=1, 62): that slot is a pad (V rows zeroed)."""
    w = l - 1
    _, extra = _window_cols(l)
    r0, r1, r2 = (int(ra_h[w, j]) for j in range(R))
    return [extra, r0, r1, r2]


def _stage_core_inputs(q, k, v, ra, pairs):
    """Build all host-staged arrays for one core (list of (b,h) pairs).

    All staged arrays are SBUF-partition-major with contiguous free dims,
    so every DMA is one large contiguous descriptor per partition."""
    P = len(pairs)
    FW = BLK + 1
    QT = np.empty((P, D, S), dtype=_BF16)
    KT = np.empty((P, D, S), dtype=_BF16)
    VA = np.empty((P, 128, 32 * FW), dtype=_BF16)   # V pairs (2c,2c+1) + ones
    VB = np.empty((P, 128, 31 * FW), dtype=_BF16)   # V pairs (2c+1,2c+2)
    VG2 = np.empty((P, 128, NW * 2 * FW), dtype=_BF16)  # [x|r0],[r1|r2] pairs
    KTr = np.empty((P, D, NW * 4 * BLK), dtype=_BF16)   # [x|r0|r1|r2] cols
    KTd0 = np.empty((P, D, 128), dtype=_BF16)             # [b0 | b63]
    QTd = np.empty((P, D, 128), dtype=_BF16)              # [q0 | q63]
    Vd0 = np.empty((P, 128, FW), dtype=_BF16)

    for i, (b, h) in enumerate(pairs):
        Q = q[b, h]; K = k[b, h]; V = v[b, h]
        qt = Q.T.astype(_BF16); kt = K.T.astype(_BF16)
        QT[i] = qt; KT[i] = kt
        vv = np.concatenate([V, np.ones((S, 1), np.float32)], 1).astype(_BF16)
        VA[i] = vv.reshape(32, 128, FW).transpose(1, 0, 2).reshape(128, -1)
        VB[i] = vv[BLK:-BLK].reshape(31, 128, FW).transpose(1, 0, 2) \
            .reshape(128, -1)
        QTd[i, :, :BLK] = qt[:, :BLK]
        QTd[i, :, BLK:] = qt[:, -BLK:]
        Vd0[i, :BLK] = vv[:BLK]
        Vd0[i, BLK:] = vv[-BLK:]
        vg = np.zeros((NW, 2, 2 * BLK, FW), np.float32)
        for l in range(1, NB - 1):
            w = l - 1
            x, r0, r1, r2 = _chunk23_blocks(l, ra[h])
            # reference's middle-block V rotation: score(k63)*V[r0],
            # score(r0)*V[r1], score(r1)*V[r2], score(r2)*V[63].
            # chunk regrouping: c0=[k0|r2]->(V0|V63)=vd0,
            # c2=[l+1|k63]->(V_{l+1}|V_r0) (K via stepped kt AP),
            # c3=[r0|r1]->(V_r1|V_r2).  Edge blocks keep the straight map:
            # c0=[k0|k63]->vd0, c2=[pad|r0]->(0|V_r0) (K_r0 via ktd0),
            # c3=[r1|r2]->(V_r1|V_r2).
            if x is None:
                kc0 = (0, NB - 1)
                kc3 = (r1, r2)
                vc2 = (None, r0)
                vc3 = (r1, r2)
                KTd0[i, :, (0 if w == 0 else 1) * BLK:
                     (1 if w == 0 else 2) * BLK] = \
                    kt[:, r0 * BLK:(r0 + 1) * BLK]
            else:
                kc0 = (0, r2)
                kc3 = (r0, r1)
                vc2 = (x, r0)
                vc3 = (r1, r2)
            for s_i, blkid in enumerate(kc0):
                KTr[i, :, w * 4 * BLK + s_i * BLK:
                    w * 4 * BLK + (s_i + 1) * BLK] = \
                    kt[:, blkid * BLK:(blkid + 1) * BLK]
            for s_i, blkid in enumerate(kc3):
                KTr[i, :, w * 4 * BLK + (2 + s_i) * BLK:
                    w * 4 * BLK + (3 + s_i) * BLK] = \
                    kt[:, blkid * BLK:(blkid + 1) * BLK]
            for ci, pair in enumerate((vc2, vc3)):
                for s_i, blkid in enumerate(pair):
                    if blkid is not None:
                        vg[w, ci, s_i * BLK:(s_i + 1) * BLK, :BLK] = \
                            V[blkid * BLK:(blkid + 1) * BLK]
                        vg[w, ci, s_i * BLK:(s_i + 1) * BLK, BLK] = 1.0
        VG2[i] = vg.astype(_BF16).transpose(2, 0, 1, 3).reshape(128, -1)
    dumA = np.zeros((P, D, 16), dtype=_BF16)
    dumB = np.zeros((P, 128, 16), dtype=_BF16)
    blobA = np.concatenate([dumA, QT, KT, KTr, KTd0, QTd], axis=2)
    blobB = np.concatenate([dumB, VA, VB, VG2, Vd0], axis=2)
    return dict(blobA=blobA, blobB=blobB)


def _build_program():
    import concourse.bass as bass
    import concourse.bacc as bacc
    import concourse.tile as tile
    from concourse import mybir

    bf16 = mybir.dt.float16  # fp16 everywhere (see _BF16)
    f32 = mybir.dt.float32
    EXP = mybir.ActivationFunctionType.Exp
    P3 = 1   # one (b,h) pair per program invocation (3 waves per core)
    FW = BLK + 1
    # Bacc (not plain Bass): its finalize() runs move_matmul_waits_to_
    # ldweights + generate_event_semaphores, which split multi-waits to
    # satisfy this walrus build's 1-wait-per-instruction constraint.
    nc = bacc.Bacc()
    # blobA (64 partitions): [QT | KT | KTr | KTd0 | QTd]
    # blobB (128 partitions): [VA | VB | VG2 | Vd0]
    # 16 dummy head columns: pair-0/1 DMAs write them (the pair-0 observer
    # DMAs read them); pair-2's DMA skips them, avoiding a WAR with the
    # observers that would add a second wait.
    CA = (S, S, NW * 4 * BLK, 128, 128)
    OA = [16]
    for c in CA:
        OA.append(OA[-1] + c)
    CB = (32 * FW, 31 * FW, NW * 2 * FW, FW)
    OB = [16]
    for c in CB:
        OB.append(OB[-1] + c)
    blobAp = nc.declare_dram_parameter("blobA", [P3, D, OA[-1]], bf16,
                                       isOutput=False)
    blobBp = nc.declare_dram_parameter("blobB", [P3, 128, OB[-1]], bf16,
                                       isOutput=False)
    outp = nc.declare_dram_parameter("out", [P3, BLK, NB * BLK], bf16,
                                     isOutput=True)
    ngroups = (NW + GROUP - 1) // GROUP  # 11 groups: 10x6 + 1x2

    with tile.TileContext(nc) as tc:
        with (
            tc.tile_pool(name="big", bufs=1) as big,      # per-pair inputs
            tc.tile_pool(name="outp", bufs=1) as outpool,  # fresh out slot
            tc.tile_pool(name="at", bufs=2) as atp,       # exp outputs
            tc.tile_pool(name="small", bufs=2) as small,
            tc.tile_pool(name="ps_s", bufs=2, space="PSUM") as ps_s,
            tc.tile_pool(name="ps_c", bufs=2, space="PSUM") as ps_c,
        ):
            # walrus codegen allows at most ONE semaphore wait per engine
            # instruction.  Every SBUF input tile gets a fresh slot (bufs=3,
            # one per pair) so DMAs carry no waits; before each phase the
            # remaining cross-engine deps are staged one-at-a-time onto
            # 1-column ldweights (PE), a self-copy (ACT), and one toucher
            # matmul (PE self-release of recycled PSUM slots), with
            # no_sync_barrier() pinning segment order.
            prev_at = None   # last SBUF tile written by ACT (bf16)
            prev_scr = None  # last SBUF tile written by DVE (bf16)
            for p in range(P3):
                # one pair per program: every SBUF tile is written exactly
                # once, so no DMA carries a slot-release or WAW wait; the
                # only dep-carrying DMA is the output (1 DVE wait, first on
                # its HWDGE queue).  The host invokes this program once per
                # pair (3 waves).
                tc.no_sync_barrier()
                ba = big.tile([D, OA[-1]], bf16, tag="ba")
                nc.sync.dma_start(out=ba[:], in_=blobAp[p])
                tc.no_sync_barrier()
                bb = big.tile([128, OB[-1]], bf16, tag="bb")
                nc.sync.dma_start(out=bb[:], in_=blobBp[p])
                tc.no_sync_barrier()
                qt = ba[:, OA[0]:OA[1]]
                kt = ba[:, OA[1]:OA[2]]
                ktr = ba[:, OA[2]:OA[3]]
                ktd0 = ba[:, OA[3]:OA[4]]
                qtd = ba[:, OA[4]:OA[5]]
                va = bb[:, OB[0]:OB[1]]
                vb = bb[:, OB[1]:OB[2]]
                vg2 = bb[:, OB[2]:OB[3]]
                vd0 = bb[:, OB[3]:OB[4]]
                # +1 scratch column: the DVE slot-acquire self-copy below
                # writes garbage there, outside the DMA'd output region.
                out_sb = outpool.tile([BLK, NB * BLK + 1], bf16, tag="out")

                # absorb the two blob-DMA completion waits, one per PE
                # inst; the DVE self-copy acquires out_sb's slot so the
                # out-DMA release wait lands here, not on the first real DVE
                # write (the garbage cell is overwritten later).
                tc.no_sync_barrier()
                nc.tensor.ldweights(ba[0:64, 0:1])
                nc.tensor.ldweights(bb[0:64, 0:1])
                nc.vector.tensor_copy(out_sb[0:1, NB * BLK:NB * BLK + 1],
                                      out_sb[0:1, 0:1])

                # ---- sparse q-blocks 1..62, in groups of GROUP ----
                for g in range(ngroups):
                    w0 = g * GROUP
                    ng = min(GROUP, NW - w0)
                    sps = ps_s.tile([128, GROUP * 256], f32, tag="s")
                    ctile = ps_c.tile([BLK, GROUP * FW + 2], f32, tag="c")
                    sc = GROUP * FW
                    tc.no_sync_barrier()
                    if prev_at is not None:
                        nc.tensor.ldweights(prev_at[0:64, 0:1])
                    if prev_scr is not None:
                        nc.tensor.ldweights(prev_scr[0:64, 0:1])
                    if prev_at is not None:
                        ascr = small.tile([1, 1], bf16, tag="ascr")
                        nc.scalar.copy(ascr[:], prev_at[0:1, 0:1])
                    tc.no_sync_barrier()
                    nc.tensor.matmul(out=ctile[0:1, sc:sc + 1],
                                     lhsT=qt[0:64, 0:1], rhs=qt[0:64, 0:1],
                                     start=True, stop=True)
                    tc.no_sync_barrier()
                    for j in range(ng):
                        l = 1 + w0 + j
                        ws, _ = _window_cols(l)
                        qcols = qt[:, l * BLK:(l + 1) * BLK]
                        base = j * 256
                        kc = (l - 1) * 4 * BLK
                        # c0: staged [K_k0|K_r2] (mid) / [K_k0|K_k63] (edge)
                        nc.tensor.matmul(
                            out=sps[:, base:base + BLK],
                            lhsT=ktr[:, kc:kc + 2 * BLK],
                            rhs=qcols, start=True, stop=True)
                        # c1: window pair from kt (contiguous)
                        nc.tensor.matmul(
                            out=sps[:, base + BLK:base + 2 * BLK],
                            lhsT=kt[:, ws * BLK:(ws + 2) * BLK],
                            rhs=qcols, start=True, stop=True)
                        # c2: mid = stepped [K_{l+1}|K_k63]; edge = dup-score
                        # top half (V zeroed) + staged K_r0 bottom half
                        if 1 < l < NW:
                            nc.tensor.matmul(
                                out=sps[0:64, base + 2 * BLK:base + 3 * BLK],
                                lhsT=kt[:, (l + 1) * BLK:(l + 2) * BLK],
                                rhs=qcols, start=True, stop=True)
                            nc.tensor.matmul(
                                out=sps[64:128, base + 2 * BLK:base + 3 * BLK],
                                lhsT=kt[:, (NB - 1) * BLK:NB * BLK],
                                rhs=qcols, start=True, stop=True)
                        else:
                            nc.tensor.matmul(
                                out=sps[0:64, base + 2 * BLK:base + 3 * BLK],
                                lhsT=kt[:, l * BLK:(l + 1) * BLK],
                                rhs=qcols, start=True, stop=True)
                            eh = 0 if l == 1 else 1
                            nc.tensor.matmul(
                                out=sps[64:128, base + 2 * BLK:base + 3 * BLK],
                                lhsT=ktd0[:, eh * BLK:(eh + 1) * BLK],
                                rhs=qcols, start=True, stop=True)
                        # c3: staged [K_r0|K_r1] (mid) / [K_r1|K_r2] (edge)
                        nc.tensor.matmul(
                            out=sps[:, base + 3 * BLK:base + 4 * BLK],
                            lhsT=ktr[:, kc + 2 * BLK:kc + 4 * BLK],
                            rhs=qcols, start=True, stop=True)

                    at = atp.tile([128, GROUP * 256], bf16, tag="at")
                    nc.scalar.activation(at[:, :ng * 256], sps[:, :ng * 256],
                                         EXP, scale=SCALE)

                    for j in range(ng):
                        l = 1 + w0 + j
                        ws, _ = _window_cols(l)
                        w = l - 1
                        base = j * 256
                        if ws % 2 == 0:
                            vwin = va[:, (ws // 2) * FW:(ws // 2 + 1) * FW]
                        else:
                            vwin = vb[:, ((ws - 1) // 2) * FW:
                                      ((ws - 1) // 2 + 1) * FW]
                        rhs = [vd0[:], vwin,
                               vg2[:, (2 * w) * FW:(2 * w + 1) * FW],
                               vg2[:, (2 * w + 1) * FW:(2 * w + 2) * FW]]
                        for c in range(4):
                            nc.tensor.matmul(
                                out=ctile[:, j * FW:(j + 1) * FW],
                                lhsT=at[:, base + c * BLK: base + (c + 1) * BLK],
                                rhs=rhs[c], start=(c == 0), stop=(c == 3))
                    csb = small.tile([BLK, GROUP * FW], f32, tag="csb")
                    nc.vector.tensor_copy(csb[:, :ng * FW], ctile[:, :ng * FW])
                    for j in range(ng):
                        l = 1 + w0 + j
                        rec = small.tile([BLK, 1], f32, tag="rec")
                        nc.vector.reciprocal(
                            rec[:], csb[:, j * FW + BLK: j * FW + BLK + 1])
                        nc.vector.tensor_scalar_mul(
                            out_sb[:, l * BLK:(l + 1) * BLK],
                            csb[:, j * FW: j * FW + BLK], rec[:, 0:1])
                    # bf16 DVE anchor: ldweights of this absorbs the DVE tick
                    scr = small.tile([BLK, 1], bf16, tag="scr")
                    nc.vector.tensor_copy(scr[:], csb[:, 0:1])
                    prev_at, prev_scr = at, scr

                # ---- dense q-blocks 0 and 63: 32 key chunks in 3 rounds ----
                cdense2 = ps_c.tile([128, FW + 2], f32, tag="c")
                tc.no_sync_barrier()
                nc.tensor.ldweights(prev_at[0:64, 0:1])
                nc.tensor.ldweights(prev_scr[0:64, 0:1])
                ascr = small.tile([1, 1], bf16, tag="ascr")
                nc.scalar.copy(ascr[:], prev_at[0:1, 0:1])
                tc.no_sync_barrier()
                nc.tensor.matmul(
                    out=cdense2[0:1, FW:FW + 1],
                    lhsT=va[0:64, 0:1], rhs=va[0:64, 0:1],
                    start=True, stop=True)
                tc.no_sync_barrier()
                # both dense blocks in ONE accumulation group: out
                # partitions 0-63 = q0 rows, 64-127 = q63 rows.  (Two
                # interleaved open groups in one PSUM bank corrupt.)
                cd = cdense2
                CH_PER = 12
                done = 0
                for rnd in range(3):
                    nch = min(CH_PER, 32 - done)
                    sps = ps_s.tile([128, GROUP * 256], f32, tag="s")
                    if rnd >= 1:
                        tc.no_sync_barrier()
                        if rnd == 2:
                            nc.tensor.ldweights(prev_at[0:64, 0:1])
                        ascr = small.tile([1, 1], bf16, tag="ascr")
                        nc.scalar.copy(ascr[:], prev_at[0:1, 0:1])
                        tc.no_sync_barrier()
                    for i in range(nch):
                        cc = done + i
                        nc.tensor.matmul(
                            out=sps[:, i * 128:(i + 1) * 128],
                            lhsT=kt[:, cc * 128:(cc + 1) * 128],
                            rhs=qtd[:], start=True, stop=True)
                    at = atp.tile([128, GROUP * 256], bf16, tag="at")
                    nc.scalar.activation(at[:, :nch * 128], sps[:, :nch * 128],
                                         EXP, scale=SCALE)
                    for i in range(nch):
                        cc = done + i
                        nc.tensor.matmul(
                            out=cd[:, 0:FW],
                            lhsT=at[:, i * 128:(i + 1) * 128],
                            rhs=va[:, cc * FW:(cc + 1) * FW],
                            start=(cc == 0), stop=(cc == 31))
                    done += nch
                    prev_at = at
                csb2 = small.tile([128, FW], f32, tag="csb2")
                nc.vector.tensor_copy(csb2[:], cd[:, :FW])
                rec2 = small.tile([128, 1], f32, tag="rec2")
                nc.vector.reciprocal(rec2[:], csb2[:, BLK:BLK + 1])
                od = small.tile([128, BLK], bf16, tag="od")
                nc.vector.tensor_scalar_mul(od[:], csb2[:, :BLK], rec2[:, 0:1])
                scr = small.tile([BLK, 1], bf16, tag="scr")
                nc.vector.tensor_copy(scr[:], csb2[0:64, 0:1])
                prev_scr = scr

                nc.sync.dma_start(out=outp[p, :, BLK:(NB - 1) * BLK],
                                  in_=out_sb[:, BLK:(NB - 1) * BLK])
                nc.sync.dma_start(out=outp[p, :, 0:BLK], in_=od[0:64, :])
                nc.sync.dma_start(out=outp[p, :, (NB - 1) * BLK:NB * BLK],
                                  in_=od[64:128, :])
    nc.finalize()
    return nc


_PROGRAM = None


def kernel(**inputs) -> np.ndarray:
    q = _np(inputs["query"]).astype(np.float32)
    k = _np(inputs["key"]).astype(np.float32)
    v = _np(inputs["value"]).astype(np.float32)
    ra = _np(inputs["random_attn"]).astype(np.int64)
    masks_ok = (
        q.shape == (B, H, S, D)
        and int(_np(inputs["q_block_size"])) == BLK
        and int(_np(inputs["kv_block_size"])) == BLK
        and np.all(_np(inputs["q_mask"]) == 1)
        and np.all(_np(inputs["kv_mask"]) == 1)
        and np.all(_np(inputs["band_mask"]) == 1)
        and np.all(_np(inputs["q_block_mask"]) == 1)
        and np.all(_np(inputs["kv_block_mask"]) == 1)
    )
    if not masks_ok:
        return _ref_numpy(
            q, k, v, _np(inputs["q_mask"]).astype(np.float32),
            _np(inputs["kv_mask"]).astype(np.float32),
            _np(inputs["band_mask"]).astype(np.float32),
            _np(inputs["q_block_mask"]).astype(np.float32),
            _np(inputs["kv_block_mask"]).astype(np.float32),
            ra, int(_np(inputs["q_block_size"])),
            int(_np(inputs["kv_block_size"])))

    try:
        return _device_kernel(q, k, v, ra)
    except Exception as e:
        sys.stderr.write(f"device kernel failed ({e!r}); numpy fallback\n")
        return _ref_numpy(
            q, k, v, _np(inputs["q_mask"]).astype(np.float32),
            _np(inputs["kv_mask"]).astype(np.float32),
            _np(inputs["band_mask"]).astype(np.float32),
            _np(inputs["q_block_mask"]).astype(np.float32),
            _np(inputs["kv_block_mask"]).astype(np.float32),
            ra, BLK, BLK)


_EXEC = None


def _get_exec():
    """Module-level cached sharded executable (run_bass_via_pjrt builds a
    fresh jit closure per call, costing a full retrace per wave)."""
    global _EXEC, _PROGRAM
    if _EXEC is not None:
        return _EXEC
    import jax
    from jax.sharding import Mesh, PartitionSpec
    try:
        from jax.experimental.shard_map import shard_map
    except ImportError:
        from jax.shard_map import shard_map
    from concourse import bass2jax, mybir

    if _PROGRAM is None:
        _PROGRAM = _build_program()
    nc = _PROGRAM
    bass2jax.install_neuronx_cc_hook()
    pname = nc.partition_id_tensor.name if nc.partition_id_tensor else None
    in_names, out_names, out_avals, zero_outs = [], [], [], []
    for alloc in nc.m.functions[0].allocations:
        if not isinstance(alloc, mybir.MemoryLocationSet):
            continue
        name = alloc.memorylocations[0].name
        if alloc.kind == "ExternalInput":
            if name != pname:
                in_names.append(name)
        elif alloc.kind == "ExternalOutput":
            out_names.append(name)
            shape = tuple(alloc.tensor_shape)
            dtype = mybir.dt.np(alloc.dtype)
            out_avals.append(jax.core.ShapedArray(shape, dtype))
            zero_outs.append(np.zeros(shape, dtype))
    n_params = len(in_names)
    n_outs = len(out_avals)
    all_names = tuple(in_names + out_names
                      + ([pname] if pname is not None else []))

    def _body(*args):
        operands = list(args)
        if pname is not None:
            operands.append(bass2jax.partition_id_tensor())
        outs = bass2jax._bass_exec_p.bind(
            *operands,
            out_avals=tuple(out_avals),
            in_names=all_names,
            out_names=tuple(out_names),
            lowering_input_output_aliases=(),
            sim_require_finite=True,
            sim_require_nnan=True,
            nc=nc,
        )
        return tuple(outs)

    devices = jax.devices()[:NCORES]
    mesh = Mesh(np.asarray(devices), ("core",))
    specs = (PartitionSpec("core"),)
    fn = jax.jit(
        shard_map(_body, mesh=mesh, in_specs=specs * (n_params + n_outs),
                  out_specs=specs * n_outs, check_rep=False),
        donate_argnums=tuple(range(n_params, n_params + n_outs)),
        keep_unused=True,
    )
    _EXEC = (fn, in_names, out_names, out_avals, zero_outs)
    return _EXEC


def _device_kernel(q, k, v, ra, trace=False):
    fn, in_names, out_names, out_avals, zero_outs = _get_exec()
    pair_list = [(b, h) for b in range(B) for h in range(H)]
    out = np.empty((B, H, S, D), dtype=np.float32)
    oi = out_names.index("out")
    # one program invocation per wave (core c handles pair 3c+w); submit all
    # waves before fetching -- jax dispatch is async, so transfers and
    # execution of successive waves overlap.
    pending = []
    for w in range(PAIRS_PER_CORE):
        in_maps = []
        wave_pairs = []
        for c in range(NCORES):
            pr = pair_list[c * PAIRS_PER_CORE + w]
            wave_pairs.append(pr)
            in_maps.append(_stage_core_inputs(q, k, v, ra, [pr]))
        concat_in = [
            np.concatenate([in_maps[c][name] for c in range(NCORES)], axis=0)
            for name in in_names
        ]
        concat_zeros = [
            np.zeros((NCORES * z.shape[0], *z.shape[1:]), z.dtype)
            for z in zero_outs
        ]
        pending.append((wave_pairs, fn(*concat_in, *concat_zeros)))
    for wave_pairs, out_arrs in pending:
        o_all = np.asarray(out_arrs[oi]) \
            .reshape(NCORES, *out_avals[oi].shape)
        for c, (b, h) in enumerate(wave_pairs):
            o = o_all[c].astype(np.float32)
            # device layout [q(64), blk(64), d] -> [blk, q, d] -> [S, D]
            out[b, h] = o[0].reshape(BLK, NB, D).transpose(1, 0, 2) \
                .reshape(S, D)
    return out

